# revision 52
# baseline (speedup 1.0000x reference)
"""Trainium2 Bass kernel for LocalRelationalLayer (sparse_attention).

Computation (per reference):
  xp = zero-pad(x, 3)                                   # [B,256,62,62]
  km = 1x1conv(xp, k_w)+k_b ; qm = 1x1conv(xp, q_w)+q_b # [B,32,.,.]
  E[b,cm,l,ky,kx] = exp(km[b,cm,r+ky,w+kx]*qm[b,cm,r+3,w+3] + gpk[cm,ky,kx])
  ck = E / sum_kx E                                     # softmax over kx only
  pre[b,m*32+cm,l] = sum_{ky,kx} ck * xp[b,m*32+cm,r+ky,w+kx]
  out = 1x1conv(pre, f_w)+f_b                           # [B,256,56,56]

Sharding: 8 cores = (b in 2) x (4 row-blocks of 14 output rows); halo rows in
the per-core slice; host concatenates. No collectives.

Per-core strategy ("all-packed", v2 schedule):
  - Attention weights in a PACKED partition layout p = cm*4+g (g = ky within
    a ky-group; group A: ky 0-3, group B: ky 4-6 + dup slot).
  - km computed on 32 partitions in 3 column chunks; cheap strided DMAs remap
    to the packed ky-shifted km4 views and per-head value views (xq).
  - Startup DMAs fan out over all four queue engines so the km matmul starts
    as soon as possible; attention phases pipeline in the order
    (A,h0), (B,h0), (A,h1), (B,h1) matching the value-block order.
  - Value phase: 32 (unit, half) jobs split DVE/Pool by their respective
    rates; the TensorEngine sums kx planes + undoes the packing via 0/1
    selection matmuls; a few jobs pre-reduce 7->3 planes on DVE to balance
    PE against the vector engines.
  - Final 1x1 conv per half as soon as that half's accumulators are done.
"""

import numpy as np
import ml_dtypes

B, C, H, W = 2, 256, 56, 56
K, PAD, M, CM = 7, 3, 8, 32
HP, WP = H + 2 * PAD, W + 2 * PAD      # 62, 62
RB = 4                                  # row blocks per batch
RH = H // RB                            # 14 output rows per core
RHP = RH + K                            # 21 stored rows per core (20 + 1 junk)
NCORES = 8
L = RH * W                              # 784 output positions per core

_bf16 = ml_dtypes.bfloat16
_PROGRAM = None

# column splits of the 784 positions into PSUM-bank-sized pieces
SPLITS = ((0, 392), (392, 392))

NKM = RHP * WP          # 1302
NQM = RH * WP           # 868
KM_CHUNKS = (0, 434, 868, 1302)     # km32 col chunks (rows 0-6 / 7-13 / 14-20)
QM_CHUNKS = (0, 434, 868)           # qm col chunks

# ---- value-phase schedule -------------------------------------------------
# Four blocks in order (grp, half); each lists 8 jobs (ci, m, eng, planes).
# eng 'v' = DVE, 'p' = Pool.  planes 3 => presum 7->3 kx on DVE first.
BLOCKS = [
    (0, 0, [(0, 0, 'v', 7), (0, 1, 'p', 7), (1, 0, 'v', 7), (0, 2, 'p', 7),
            (1, 1, 'v', 7), (0, 3, 'p', 7), (1, 2, 'v', 7), (1, 3, 'p', 7)]),
    (1, 0, [(0, 0, 'v', 7), (0, 1, 'p', 7), (1, 0, 'v', 7), (0, 2, 'p', 7),
            (1, 1, 'v', 7), (0, 3, 'p', 7), (1, 2, 'v', 7), (1, 3, 'p', 4)]),
    (0, 1, [(0, 0, 'v', 7), (0, 1, 'p', 7), (1, 0, 'v', 7), (0, 2, 'v', 4),
            (1, 1, 'p', 7), (0, 3, 'p', 7), (1, 2, 'p', 7), (1, 3, 'v', 4)]),
    (1, 1, [(0, 0, 'v', 7), (0, 1, 'p', 7), (1, 0, 'v', 4), (0, 2, 'v', 4),
            (1, 1, 'p', 7), (0, 3, 'v', 4), (1, 2, 'p', 7), (1, 3, 'v', 4)]),
]

# DVE-queue injections of recip+norm for later phases: after (block, job).
NORM_INJECT = {(0, 3): (1, 0), (1, 1): (0, 1), (1, 5): (1, 1)}
# PE-queue injections of the dsum matmuls for the h1 phases.
DSUM_INJECT = {(0, 5): (0, 1), (1, 3): (1, 1)}
# P4 engine split: kx < P4_NSPLIT on DVE, rest on Pool (per phase)
P4_NSPLIT = {(0, 0): 4, (1, 0): 3, (0, 1): 3, (1, 1): 2}
# engine for each phase's ck normalize multiply ('v' DVE / 'p' Pool)
NORM_ENG = {(0, 0): 's', (1, 0): 'v', (0, 1): 'v', (1, 1): 'v'}
CONV_PIECES = ((0, 392),)              # h1 y-write pieces
CONV_H0_ACT = True                     # h0 ci1 pre-copy on Act


def _build_program():
    import concourse.bass as bass
    import concourse.tile as tile
    from concourse import bacc, mybir
    from concourse.ap import AP

    f32 = mybir.dt.float32
    bf16 = mybir.dt.bfloat16
    Exp = mybir.ActivationFunctionType.Exp
    Ident = mybir.ActivationFunctionType.Identity
    PS = bass.MemorySpace.PSUM

    nc = bacc.Bacc("TRN2", target_bir_lowering=False, debug=False,
                   num_devices=NCORES)

    xp_d = nc.dram_tensor("xp", [128, 2, RHP, WP], bf16, kind="ExternalInput")
    wk_d = nc.dram_tensor("wk", [128, 2, CM], bf16, kind="ExternalInput")
    wq_d = nc.dram_tensor("wq", [128, 2, 128], bf16, kind="ExternalInput")
    sm_d = nc.dram_tensor("sm", [128, 9, 128], bf16, kind="ExternalInput")
    fw_d = nc.dram_tensor("fw", [128, 4, 128], bf16, kind="ExternalInput")
    gpk_d = nc.dram_tensor("gpk", [128, 2 * K], f32, kind="ExternalInput")
    kb_d = nc.dram_tensor("kb", [CM, 1], f32, kind="ExternalInput")
    qb_d = nc.dram_tensor("qb", [128, 1], f32, kind="ExternalInput")
    fb_d = nc.dram_tensor("fb", [128, 2], f32, kind="ExternalInput")
    y_d = nc.dram_tensor("y", [128, 2, L], bf16, kind="ExternalOutput")

    XSPLIT = 11            # xv row split for the two DMA chunks per ci

    with tile.TileContext(nc) as tc:
        with (
            tc.tile_pool(name="inp", bufs=1) as inp,
            tc.tile_pool(name="wpool", bufs=1) as wpool,
            tc.tile_pool(name="kq", bufs=1) as kq,
            tc.tile_pool(name="att", bufs=1) as att,
            tc.tile_pool(name="pv", bufs=8) as pvp,
            tc.tile_pool(name="pvw", bufs=2) as pvw,
            tc.tile_pool(name="outp", bufs=1) as outp,
            tc.tile_pool(name="psMM", bufs=2, space=PS) as psMM,
            tc.tile_pool(name="psA", bufs=1, space=PS) as psA,
            tc.tile_pool(name="psB", bufs=2, space=PS) as psB,
        ):
            # ---------------- startup DMAs on all four queues ------------
            xvA = inp.tile([128, 2, XSPLIT, WP], bf16, tag="xvA", name="xvA")
            xvB = inp.tile([128, 2, RHP - XSPLIT, WP], bf16, tag="xvB",
                           name="xvB")
            # Act queue: small weights (kb/qb/gpk)
            kb = wpool.tile([CM, 1], f32, tag="kb", name="kb")
            nc.scalar.dma_start(kb[:], kb_d.ap())
            qb = wpool.tile([128, 1], f32, tag="qb", name="qb")
            nc.scalar.dma_start(qb[:], qb_d.ap())
            gpk = wpool.tile([128, 2 * K], f32, tag="gpk", name="gpk")
            nc.scalar.dma_start(gpk[:], gpk_d.ap())
            scratch = wpool.tile([32, 1], f32, tag="scr", name="scr")
            nc.gpsimd.memset(scratch[:], 0.0)
            warm_sb = wpool.tile([32, 512], bf16, tag="warm", name="warm")
            nc.gpsimd.memset(warm_sb[:], 0.0)

            # Pool queue: wk + xvA ci0 (Pool engine-time is precious)
            wk = wpool.tile([128, 2, CM], bf16, tag="wk", name="wk")
            nc.gpsimd.dma_start(wk[:], wk_d.ap())
            nc.gpsimd.dma_start(xvA[:][:, 0], xp_d.ap()[:, 0, 0:XSPLIT])
            # wq on Pool (needed by the qm matmuls ~4.5us)
            wq = wpool.tile([128, 2, 128], bf16, tag="wq", name="wq")
            nc.gpsimd.dma_start(wq[:], wq_d.ap())

            # SP queue: xv ci1 chunks + xvB ci0, sm, then remaps.
            nc.sync.dma_start(xvA[:][:, 1], xp_d.ap()[:, 1, 0:XSPLIT])
            nc.sync.dma_start(xvB[:][:, 0], xp_d.ap()[:, 0, XSPLIT:])
            nc.sync.dma_start(xvB[:][:, 1], xp_d.ap()[:, 1, XSPLIT:])
            smat = wpool.tile([128, 9, 128], bf16, tag="sm", name="sm")

            # xq value-view remaps read DRAM directly -> no deps, issue early
            xq = [[None] * 4 for _ in range(4)]  # [grp*2+ci][mslot]

            def emit_xq(grp, ci, mslot):
                base = 0 if grp == 0 else 4
                t = kq.tile([128, RH, WP], bf16,
                            tag=f"xq{grp}{ci}{mslot}",
                            name=f"xq{grp}{ci}{mslot}")
                src0 = xp_d.ap()[mslot * 32:(mslot + 1) * 32, ci]
                part = list(src0.ap[0])
                src = AP(tensor=src0.tensor,
                         offset=src0.offset + base * WP,
                         ap=[part, [WP, 4], [WP, RH], [1, WP]])
                nc.sync.dma_start(t[:], src)
                xq[grp * 2 + ci][mslot] = t

            fb = wpool.tile([128, 2], f32, tag="fb", name="fb")

            # ---------------- km32 / qm4 matmuls -------------------------
            # three overlapping row-range bf16 copies of km:
            #   aa = rows 0-9   (feeds km4(0,0))
            #   ab = rows 4-13  (feeds km4(1,0))
            #   b  = rows 7-20  (feeds km4(0,1) and km4(1,1))
            km32aa = kq.tile([CM, 10, WP], bf16, tag="km32aa", name="km32aa")
            km32ab = kq.tile([CM, 10, WP], bf16, tag="km32ab", name="km32ab")
            km32b = kq.tile([CM, 2 * K, WP], bf16, tag="km32b", name="km32b")
            aa_f = km32aa[:].rearrange("p r w -> p (r w)")
            ab_f = km32ab[:].rearrange("p r w -> p (r w)")
            b_f = km32b[:].rearrange("p r w -> p (r w)")
            qm4h = [kq.tile([128, K, WP], bf16, tag=f"qm4h{h}",
                            name=f"qm4h{h}") for h in range(2)]
            xvA_f = xvA[:].rearrange("p c r w -> p (c r w)")
            xvB_f = xvB[:].rearrange("p c r w -> p (c r w)")
            NA = XSPLIT * WP           # 682
            NB = (RHP - XSPLIT) * WP   # 620

            # PE p-state warmup: junk matmuls on zeroed SBUF ramp the clock
            warm_ps = psMM.tile([128, 512], f32, tag="mm", name="warm")
            for _ in range(5):
                nc.tensor.matmul(warm_ps[:], warm_sb[:, 0:128], warm_sb[:],
                                 start=True, stop=True)
            wjunk = wpool.tile([1, 8], f32, tag="wj", name="wj")
            nc.scalar.copy(wjunk[:], warm_ps[:1, :8])

            def km_chunk(name, src_f, off, n):
                ps = psMM.tile([128, 512], f32, tag="mm", name=name)
                for ci in range(2):
                    nc.tensor.matmul(
                        ps[:CM, :n],
                        wk[:, ci], src_f[:, ci * (NA if src_f is xvA_f else NB)
                                         + off: ci * (NA if src_f is xvA_f
                                                      else NB) + off + n],
                        start=(ci == 0), stop=(ci == 1))
                return ps

            ps_c0 = km_chunk("km_c0", xvA_f, 0, 434)        # rows 0-6
            ps_c1 = km_chunk("km_c1", xvA_f, 434, 248)      # rows 7-10
            # aa copies (rows 0-9) -> km4(0,0) can go as soon as these land
            nc.scalar.activation(aa_f[:, 0:434], ps_c0[:CM, :434],
                                 Ident, bias=kb[:], scale=1.0)
            nc.scalar.activation(aa_f[:, 434:620], ps_c1[:CM, :186],
                                 Ident, bias=kb[:], scale=1.0)
            ps_c2 = km_chunk("km_c2", xvB_f, 0, 434)        # rows 11-17
            # ab copies (rows 4-13)
            nc.scalar.activation(ab_f[:, 0:186], ps_c0[:CM, 248:434],
                                 Ident, bias=kb[:], scale=1.0)
            nc.scalar.activation(ab_f[:, 186:434], ps_c1[:CM, :248],
                                 Ident, bias=kb[:], scale=1.0)
            nc.scalar.activation(ab_f[:, 434:620], ps_c2[:CM, :186],
                                 Ident, bias=kb[:], scale=1.0)
            ps_c3 = km_chunk("km_c3", xvB_f, 434, 186)      # rows 18-20
            # b copies (rows 7-20)
            nc.scalar.activation(b_f[:, 0:248], ps_c1[:CM, :248],
                                 Ident, bias=kb[:], scale=1.0)
            nc.scalar.activation(b_f[:, 248:682], ps_c2[:CM, :434],
                                 Ident, bias=kb[:], scale=1.0)
            nc.scalar.activation(b_f[:, 682:868], ps_c3[:CM, :186],
                                 Ident, bias=kb[:], scale=1.0)
            # qm chunks after all km chunks (wq arrives later than wk)
            psq0 = psMM.tile([128, 512], f32, tag="mm", name="psq0")
            for ci in range(2):
                nc.tensor.matmul(psq0[:, :434],
                                 wq[:, ci],
                                 xvA_f[:, ci * NA + PAD * WP:
                                       ci * NA + PAD * WP + 434],
                                 start=(ci == 0), stop=(ci == 1))
            psq1 = psMM.tile([128, 512], f32, tag="mm", name="psq1")
            for ci in range(2):
                nc.tensor.matmul(psq1[:, 0:62],
                                 wq[:, ci], xvA_f[:, ci * NA + 620:
                                                  ci * NA + 682],
                                 start=(ci == 0), stop=False)
                nc.tensor.matmul(psq1[:, 62:434],
                                 wq[:, ci], xvB_f[:, ci * NB: ci * NB + 372],
                                 start=False, stop=(ci == 1))

            # exp-table preload once the Act queue head has drained
            nc.scalar.activation(scratch[:], scratch[:], Exp, bias=0.0,
                                 scale=1.0)

            # qm bias-copies on DVE
            nc.vector.tensor_scalar_add(
                qm4h[0][:].rearrange("p r w -> p (r w)"),
                psq0[:, :434], qb[:])
            nc.vector.tensor_scalar_add(
                qm4h[1][:].rearrange("p r w -> p (r w)"),
                psq1[:, :434], qb[:])

            # km4 remap DMAs, one tile per (grp, half)
            km4h = [[kq.tile([128, K, WP], bf16, tag=f"km4{g}{h}",
                             name=f"km4{g}{h}") for h in range(2)]
                    for g in range(2)]

            def emit_km4(grp, h, queue):
                # source tile and local base row for each (grp, h):
                # (0,0)->aa row 0; (1,0)->ab row 0; (0,1)->b row 0;
                # (1,1)->b row 4
                srcs = {(0, 0): (km32aa, 0), (1, 0): (km32ab, 0),
                        (0, 1): (km32b, 0), (1, 1): (km32b, 4)}
                tile_src, base = srcs[(grp, h)]
                a = tile_src[:]
                part = list(a.ap[0])
                src = AP(tensor=a.tensor, offset=a.offset + base * WP,
                         ap=[part, [WP, 4], [WP, K], [1, WP]])
                queue.dma_start(km4h[grp][h][:], src)

            emit_km4(0, 0, nc.sync)
            emit_km4(1, 0, nc.sync)
            emit_km4(0, 1, nc.sync)
            emit_km4(1, 1, nc.sync)
            nc.sync.dma_start(smat[:], sm_d.ap())
            nc.sync.dma_start(fb[:], fb_d.ap())
            # xq remaps (after the km4 remaps on SP)
            for grp, h, jobs in BLOCKS[:2]:
                for (ci, m, eng, planes) in jobs:
                    if xq[grp * 2 + ci][m] is None:
                        emit_xq(grp, ci, m)

            # fw needed only for the final conv
            fw = wpool.tile([128, 4, 128], bf16, tag="fw", name="fw")
            nc.sync.dma_start(fw[:], fw_d.ap())

            ident = smat[:, 8]               # [128, 128] identity

            # ---------------- attention (packed, per (grp, half)) --------
            P4h = [[att.tile([128, K, K, W], bf16, tag=f"P4{g}{h}",
                             name=f"P4{g}{h}") for h in range(2)]
                   for g in range(2)]
            E4h = [[att.tile([128, K, 392], bf16, tag=f"E4{g}{h}",
                             name=f"E4{g}{h}") for h in range(2)]
                   for g in range(2)]
            dps = [[None, None] for _ in range(2)]
            rbh = [[att.tile([128, 392], bf16, tag=f"rb{g}{h}",
                             name=f"rb{g}{h}") for h in range(2)]
                   for g in range(2)]
            ck4h = [[att.tile([128, K, 392], bf16, tag=f"ck{g}{h}",
                              name=f"ck{g}{h}") for h in range(2)]
                    for g in range(2)]

            def emit_P4(grp, h):
                nsplit = P4_NSPLIT[(grp, h)]
                qmc = qm4h[h][:][:, :, PAD:PAD + W]
                for kx in range(K):
                    eng = nc.vector if kx < nsplit else nc.gpsimd
                    eng.tensor_mul(
                        P4h[grp][h][:, kx],
                        km4h[grp][h][:][:, :, kx:kx + W],
                        qmc)

            def emit_exps(grp, h):
                for kx in range(K):
                    nc.scalar.activation(
                        E4h[grp][h][:, kx],
                        P4h[grp][h][:, kx].rearrange("p r w -> p (r w)"),
                        Exp,
                        bias=gpk[:, grp * K + kx:grp * K + kx + 1],
                        scale=1.0)

            def emit_dsum(grp, h):
                ps = psB.tile([128, 392], f32, tag="pso", name=f"d{grp}{h}")
                dps[grp][h] = ps
                for kx in range(K):
                    nc.tensor.matmul(ps[:], ident, E4h[grp][h][:, kx],
                                     start=(kx == 0), stop=(kx == K - 1))

            def emit_norm(grp, h):
                from concourse.dve_ops import (RECIPROCAL_APPROX_FAST,
                                               RECIP_APPROX_FAST_CONSTS)
                dsrc = dps[grp][h][:]
                nc.vector._custom_dve(RECIPROCAL_APPROX_FAST,
                                      out=rbh[grp][h][:], in0=dsrc,
                                      **RECIP_APPROX_FAST_CONSTS)
                ne = NORM_ENG[(grp, h)]
                if ne == 's':   # split across both vector engines
                    nc.vector.tensor_mul(
                        ck4h[grp][h][:][:, 0:4], E4h[grp][h][:][:, 0:4],
                        rbh[grp][h][:].unsqueeze(1).broadcast_to((128, 4, 392)))
                    nc.gpsimd.tensor_mul(
                        ck4h[grp][h][:][:, 4:7], E4h[grp][h][:][:, 4:7],
                        rbh[grp][h][:].unsqueeze(1).broadcast_to((128, 3, 392)))
                else:
                    neng = nc.vector if ne == 'v' else nc.gpsimd
                    neng.tensor_mul(
                        ck4h[grp][h][:], E4h[grp][h][:],
                        rbh[grp][h][:].unsqueeze(1).broadcast_to((128, K, 392)))

            # attention phases in value-block order
            for (grp, h) in [(0, 0), (1, 0), (0, 1), (1, 1)]:
                emit_P4(grp, h)
                emit_exps(grp, h)
            emit_dsum(0, 0)
            if (1, 0) not in DSUM_INJECT.values():
                emit_dsum(1, 0)
            emit_norm(0, 0)

            # ---------------- value phase --------------------------------
            pre_ps = [[psA.tile([128, n], f32, tag=f"pre{ci}{si}",
                                name=f"pre{ci}{si}")
                       for si, (o, n) in enumerate(SPLITS)] for ci in range(2)]
            first = [[True] * 2 for _ in range(2)]
            # last (ci, h) job position for stop flags
            last_pos = {}
            for bi, (grp, h, jobs) in enumerate(BLOCKS):
                for ji, (ci, m, eng, planes) in enumerate(jobs):
                    last_pos[(ci, h)] = (bi, ji)

            def emit_job(bi, ji, grp, h, ci, m, eng_c, planes):
                PV = pvp.tile([128, K, 392], bf16, tag="PV",
                              name=f"PV{bi}_{ji}")
                eng = nc.vector if eng_c == 'v' else nc.gpsimd
                xw = xq[grp * 2 + ci][m][:]
                part = list(xw.ap[0])
                win = AP(tensor=xw.tensor,
                         offset=xw.offset + (7 * WP if h else 0),
                         ap=[part, [1, K], [WP, K], [1, W]])
                eng.tensor_mul(
                    PV[:].rearrange("p k (r w) -> p k r w", r=K),
                    ck4h[grp][h][:].rearrange("p k (r w) -> p k r w", r=K),
                    win)
                S_ap = smat[:, grp * 4 + m]
                if planes == 4:
                    # single-inst 7->4 presum: PW[j] = PV[j] + PV[j+3]
                    PW = pvw.tile([128, 3, 392], bf16, tag="PW",
                                  name=f"PW{bi}_{ji}")
                    eng.tensor_add(PW[:], PV[:, 0:3], PV[:, 3:6])
                    srcs = [PW[:, 0, :], PW[:, 1, :], PW[:, 2, :],
                            PV[:, 6, :]]
                else:
                    srcs = [PV[:, kx, :] for kx in range(K)]
                for kx, s in enumerate(srcs):
                    is_last = (last_pos[(ci, h)] == (bi, ji)
                               and kx == len(srcs) - 1)
                    nc.tensor.matmul(pre_ps[ci][h][:], S_ap, s,
                                     start=first[ci][h], stop=is_last)
                    first[ci][h] = False

            # ---------------- final 1x1 conv (per half) ------------------
            y_sb = outp.tile([128, 2, L], bf16, tag="ysb", name="ysb")
            pre_sb = [[None] * 2 for _ in range(2)]

            def emit_conv(h):
                o, n = SPLITS[h]
                for ci in range(2):
                    t = outp.tile([128, 392], bf16, tag=f"psb{ci}{h}",
                                  name=f"psb{ci}{h}")
                    pre_sb[ci][h] = t
                    if ci == 0 or (h == 0 and CONV_H0_ACT):
                        nc.scalar.copy(t[:], pre_ps[ci][h][:])
                    else:
                        nc.vector.tensor_scalar_mul(t[:], pre_ps[ci][h][:],
                                                    1.0)
                # two column pieces per oc: bias+DMA of piece 0 overlaps the
                # rest so only a small piece trails the final matmul.
                pieces = CONV_PIECES if h == 1 else ((0, 392),)
                for oc in range(2):
                    ps = psB.tile([128, 512], f32, tag="pso",
                                  name=f"ps_o{oc}{h}")
                    for ci in range(2):
                        nc.tensor.matmul(ps[:, :n], fw[:, 2 * ci + oc],
                                         pre_sb[ci][h][:],
                                         start=(ci == 0), stop=(ci == 1))
                    for (po, pn) in pieces:
                        # h1: oc1 is gated later, so it gets the faster Act
                        # bias; oc0 takes DVE.  h0 keeps the original split.
                        oc_on_act = (oc == 1) if h == 1 else (oc == 0)
                        if oc_on_act:
                            nc.scalar.activation(
                                y_sb[:, oc, o + po:o + po + pn],
                                ps[:, po:po + pn], Ident,
                                bias=fb[:, oc:oc + 1], scale=1.0)
                            nc.sync.dma_start(
                                y_d.ap()[:, oc, o + po:o + po + pn],
                                y_sb[:][:, oc, o + po:o + po + pn])
                        else:
                            nc.vector.tensor_scalar_add(
                                y_sb[:, oc, o + po:o + po + pn],
                                ps[:, po:po + pn], fb[:, oc:oc + 1])
                            nc.scalar.dma_start(
                                y_d.ap()[:, oc, o + po:o + po + pn],
                                y_sb[:][:, oc, o + po:o + po + pn])

            for bi, (grp, h, jobs) in enumerate(BLOCKS):
                if bi == 2:
                    # grp-1 xq tiles emitted once block A0's are all queued
                    for g2, h2, jobs2 in BLOCKS[2:]:
                        for (ci, m, eng, planes) in jobs2:
                            if xq[g2 * 2 + ci][m] is None:
                                emit_xq(g2, ci, m)
                for ji, (ci, m, eng_c, planes) in enumerate(jobs):
                    emit_job(bi, ji, grp, h, ci, m, eng_c, planes)
                    if (bi, ji) in DSUM_INJECT:
                        g2, h2 = DSUM_INJECT[(bi, ji)]
                        emit_dsum(g2, h2)
                    if (bi, ji) in NORM_INJECT:
                        g2, h2 = NORM_INJECT[(bi, ji)]
                        emit_norm(g2, h2)
                # end of blocks 1 and 3 complete halves 0 and 1
                if bi == 1:
                    emit_conv(0)
                if bi == 3:
                    emit_conv(1)

    nc.compile()
    return nc


def _get_program():
    global _PROGRAM
    if _PROGRAM is None:
        _PROGRAM = _build_program()
    return _PROGRAM


def _gpk_host(gp_w1, gp_b1, gp_w2, gp_b2):
    """GeometryPrior on host (tiny: 49 positions through a 2->16->32 MLP)."""
    a = np.arange(-(K // 2), K // 2 + 1, dtype=np.float32)
    x_pos = np.broadcast_to(a[None, :], (K, K))
    y_pos = np.broadcast_to(a[::-1][:, None], (K, K))
    pos = np.stack([x_pos, y_pos], 0).astype(np.float32)          # [2,7,7]
    h1 = np.einsum('pij,mp->mij', pos, np.asarray(gp_w1, np.float32))
    h1 = np.maximum(h1 + np.asarray(gp_b1, np.float32)[:, None, None], 0.0)
    gpk = np.einsum('mij,cm->cij', h1, np.asarray(gp_w2, np.float32))
    gpk = gpk + np.asarray(gp_b2, np.float32)[:, None, None]      # [32,7,7]
    return gpk


def make_inputs(x, k_w, k_b, q_w, q_b, gp_w1, gp_b1, gp_w2, gp_b2, f_w, f_b):
    """Returns per-core input maps (list of 8 dicts)."""
    x = np.asarray(x, np.float32)
    xp = np.zeros((B, C, HP, WP), np.float32)
    xp[:, :, PAD:PAD + H, PAD:PAD + W] = x

    # channel order: chunk ci partition p -> c = (4ci + p//32)*32 + p%32
    chan = np.arange(128)
    c_of = [((4 * ci + chan // 32) * 32 + chan % 32) for ci in range(2)]

    k_w = np.asarray(k_w, np.float32)
    q_w = np.asarray(q_w, np.float32)
    f_w = np.asarray(f_w, np.float32)

    # wk[p, ci, cm] = k_w[cm, c_of[ci][p]]
    wk = np.stack([k_w[:, c_of[ci]].T for ci in range(2)], 1).astype(_bf16)
    # wq[p, ci, cm*4+g] = q_w[cm, c_of[ci][p]]
    wq_rows = np.stack([q_w[:, c_of[ci]].T for ci in range(2)], 1)  # [128,2,32]
    wq = np.repeat(wq_rows, 4, axis=2)                              # cm*4+g
    wq = wq.reshape(128, 2, CM, 4).reshape(128, 2, 128).astype(_bf16)

    # selection matrices: sm[k, grp*4+mslot, q] = 1 iff k = (q%32)*4+g valid g
    # slot 8 is the identity (for the softmax-denominator sums on PE)
    sm = np.zeros((128, 9, 128), np.float32)
    for grp in range(2):
        ng = 4 if grp == 0 else 3
        for mslot in range(4):
            for cm in range(CM):
                for g in range(ng):
                    sm[cm * 4 + g, grp * 4 + mslot, mslot * 32 + cm] = 1.0
    sm[:, 8, :] = np.eye(128, dtype=np.float32)
    sm = sm.astype(_bf16)

    # fw[p, 2*ci+oc, q] = f_w[c_of[oc][q], c_of[ci][p]]
    fw = np.zeros((128, 4, 128), np.float32)
    for ci in range(2):
        for oc in range(2):
            fw[:, 2 * ci + oc, :] = f_w[np.ix_(c_of[oc], c_of[ci])].T
    fw = fw.astype(_bf16)

    gpk_full = _gpk_host(gp_w1, gp_b1, gp_w2, gp_b2)      # [32,7,7]
    gpk = np.zeros((128, 2 * K), np.float32)
    for cm in range(CM):
        for g in range(4):
            gpk[cm * 4 + g, 0:K] = gpk_full[cm, g]
            gpk[cm * 4 + g, K:2 * K] = gpk_full[cm, 4 + min(g, 2)]

    kb = np.asarray(k_b, np.float32).reshape(CM, 1)
    qb = np.repeat(np.asarray(q_b, np.float32), 4).reshape(128, 1)
    fb = np.stack([np.asarray(f_b, np.float32)[c_of[oc]] for oc in range(2)],
                  1)                                       # [128, 2]

    xp16 = xp.astype(_bf16)
    in_maps = []
    for core in range(NCORES):
        b, rb = divmod(core, RB)
        sl = np.zeros((128, 2, RHP, WP), _bf16)
        r0 = rb * RH
        nrows = min(RHP, HP - r0)
        for ci in range(2):
            sl[:, ci, :nrows] = xp16[b, c_of[ci], r0:r0 + nrows, :]
        in_maps.append({
            "xp": sl, "wk": wk, "wq": wq, "sm": sm, "fw": fw, "gpk": gpk,
            "kb": kb, "qb": qb, "fb": fb,
        })
    return in_maps


def assemble(results):
    chan = np.arange(128)
    out = np.empty((B, C, H, W), np.float32)
    for core in range(NCORES):
        b, rb = divmod(core, RB)
        y = np.asarray(results[core]["y"], np.float32)     # [128, 2, 784]
        for oc in range(2):
            c_of = (4 * oc + chan // 32) * 32 + chan % 32
            out[b, c_of, rb * RH:(rb + 1) * RH, :] = y[:, oc].reshape(
                128, RH, W)
    return out


def kernel(**inputs):
    from concourse import bass_utils
    nc = _get_program()
    in_maps = make_inputs(**inputs)
    res = bass_utils.run_bass_kernel_spmd(nc, in_maps, list(range(NCORES)))
    return assemble(res.results)


# revision 53
# speedup vs baseline: 1.0004x; 1.0004x over previous
"""Trainium2 Bass kernel for LocalRelationalLayer (sparse_attention).

Computation (per reference):
  xp = zero-pad(x, 3)                                   # [B,256,62,62]
  km = 1x1conv(xp, k_w)+k_b ; qm = 1x1conv(xp, q_w)+q_b # [B,32,.,.]
  E[b,cm,l,ky,kx] = exp(km[b,cm,r+ky,w+kx]*qm[b,cm,r+3,w+3] + gpk[cm,ky,kx])
  ck = E / sum_kx E                                     # softmax over kx only
  pre[b,m*32+cm,l] = sum_{ky,kx} ck * xp[b,m*32+cm,r+ky,w+kx]
  out = 1x1conv(pre, f_w)+f_b                           # [B,256,56,56]

Sharding: 8 cores = (b in 2) x (4 row-blocks of 14 output rows); halo rows in
the per-core slice; host concatenates. No collectives.

Per-core strategy ("all-packed", v2 schedule):
  - Attention weights in a PACKED partition layout p = cm*4+g (g = ky within
    a ky-group; group A: ky 0-3, group B: ky 4-6 + dup slot).
  - km computed on 32 partitions in 3 column chunks; cheap strided DMAs remap
    to the packed ky-shifted km4 views and per-head value views (xq).
  - Startup DMAs fan out over all four queue engines so the km matmul starts
    as soon as possible; attention phases pipeline in the order
    (A,h0), (B,h0), (A,h1), (B,h1) matching the value-block order.
  - Value phase: 32 (unit, half) jobs split DVE/Pool by their respective
    rates; the TensorEngine sums kx planes + undoes the packing via 0/1
    selection matmuls; a few jobs pre-reduce 7->3 planes on DVE to balance
    PE against the vector engines.
  - Final 1x1 conv per half as soon as that half's accumulators are done.
"""

import numpy as np
import ml_dtypes

B, C, H, W = 2, 256, 56, 56
K, PAD, M, CM = 7, 3, 8, 32
HP, WP = H + 2 * PAD, W + 2 * PAD      # 62, 62
RB = 4                                  # row blocks per batch
RH = H // RB                            # 14 output rows per core
RHP = RH + K                            # 21 stored rows per core (20 + 1 junk)
NCORES = 8
L = RH * W                              # 784 output positions per core

_bf16 = ml_dtypes.bfloat16
_PROGRAM = None

# column splits of the 784 positions into PSUM-bank-sized pieces
SPLITS = ((0, 392), (392, 392))

NKM = RHP * WP          # 1302
NQM = RH * WP           # 868
KM_CHUNKS = (0, 434, 868, 1302)     # km32 col chunks (rows 0-6 / 7-13 / 14-20)
QM_CHUNKS = (0, 434, 868)           # qm col chunks

# ---- value-phase schedule -------------------------------------------------
# Four blocks in order (grp, half); each lists 8 jobs (ci, m, eng, planes).
# eng 'v' = DVE, 'p' = Pool.  planes 3 => presum 7->3 kx on DVE first.
BLOCKS = [
    (0, 0, [(0, 0, 'v', 7), (0, 1, 'p', 7), (1, 0, 'v', 7), (0, 2, 'p', 7),
            (1, 1, 'v', 7), (0, 3, 'p', 7), (1, 2, 'v', 7), (1, 3, 'p', 7)]),
    (1, 0, [(0, 0, 'v', 7), (0, 1, 'p', 7), (1, 0, 'v', 7), (0, 2, 'p', 7),
            (1, 1, 'v', 7), (0, 3, 'p', 7), (1, 2, 'v', 7), (1, 3, 'p', 4)]),
    (0, 1, [(0, 0, 'v', 7), (0, 1, 'p', 7), (1, 0, 'v', 7), (0, 2, 'v', 4),
            (1, 1, 'p', 7), (0, 3, 'p', 7), (1, 2, 'p', 7), (1, 3, 'v', 4)]),
    (1, 1, [(0, 0, 'v', 7), (0, 1, 'p', 7), (1, 0, 'v', 4), (0, 2, 'v', 4),
            (1, 1, 'p', 7), (0, 3, 'v', 4), (1, 2, 'p', 7), (1, 3, 'v', 4)]),
]

# DVE-queue injections of recip+norm for later phases: after (block, job).
NORM_INJECT = {(0, 3): (1, 0), (1, 1): (0, 1), (1, 5): (1, 1)}
# PE-queue injections of the dsum matmuls for the h1 phases.
DSUM_INJECT = {(0, 5): (0, 1), (1, 3): (1, 1)}
# P4 engine split: kx < P4_NSPLIT on DVE, rest on Pool (per phase)
P4_NSPLIT = {(0, 0): 4, (1, 0): 3, (0, 1): 3, (1, 1): 2}
# engine for each phase's ck normalize multiply ('v' DVE / 'p' Pool)
NORM_ENG = {(0, 0): 's', (1, 0): 'v', (0, 1): 'v', (1, 1): 'v'}
CONV_PIECES = ((0, 392),)              # h1 y-write pieces
CONV_H0_ACT = True                     # h0 ci1 pre-copy on Act


def _build_program():
    import concourse.bass as bass
    import concourse.tile as tile
    from concourse import bacc, mybir
    from concourse.ap import AP

    f32 = mybir.dt.float32
    bf16 = mybir.dt.bfloat16
    Exp = mybir.ActivationFunctionType.Exp
    Ident = mybir.ActivationFunctionType.Identity
    PS = bass.MemorySpace.PSUM

    nc = bacc.Bacc("TRN2", target_bir_lowering=False, debug=False,
                   num_devices=NCORES)

    xp_d = nc.dram_tensor("xp", [128, 2, RHP, WP], bf16, kind="ExternalInput")
    wk_d = nc.dram_tensor("wk", [128, 2, CM], bf16, kind="ExternalInput")
    wq_d = nc.dram_tensor("wq", [128, 2, 128], bf16, kind="ExternalInput")
    sm_d = nc.dram_tensor("sm", [128, 9, 128], bf16, kind="ExternalInput")
    fw_d = nc.dram_tensor("fw", [128, 4, 128], bf16, kind="ExternalInput")
    gpk_d = nc.dram_tensor("gpk", [128, 2 * K], f32, kind="ExternalInput")
    kb_d = nc.dram_tensor("kb", [CM, 1], f32, kind="ExternalInput")
    qb_d = nc.dram_tensor("qb", [128, 1], f32, kind="ExternalInput")
    fb_d = nc.dram_tensor("fb", [128, 2], f32, kind="ExternalInput")
    y_d = nc.dram_tensor("y", [128, 2, L], bf16, kind="ExternalOutput")

    XSPLIT = 11            # xv row split for the two DMA chunks per ci

    with tile.TileContext(nc) as tc:
        with (
            tc.tile_pool(name="inp", bufs=1) as inp,
            tc.tile_pool(name="wpool", bufs=1) as wpool,
            tc.tile_pool(name="kq", bufs=1) as kq,
            tc.tile_pool(name="att", bufs=1) as att,
            tc.tile_pool(name="pv", bufs=8) as pvp,
            tc.tile_pool(name="pvw", bufs=2) as pvw,
            tc.tile_pool(name="outp", bufs=1) as outp,
            tc.tile_pool(name="psMM", bufs=2, space=PS) as psMM,
            tc.tile_pool(name="psA", bufs=1, space=PS) as psA,
            tc.tile_pool(name="psB", bufs=2, space=PS) as psB,
        ):
            # ---------------- startup DMAs on all four queues ------------
            xvA = inp.tile([128, 2, XSPLIT, WP], bf16, tag="xvA", name="xvA")
            xvB = inp.tile([128, 2, RHP - XSPLIT, WP], bf16, tag="xvB",
                           name="xvB")
            # Act queue: small weights (kb/qb/gpk)
            kb = wpool.tile([CM, 1], f32, tag="kb", name="kb")
            nc.scalar.dma_start(kb[:], kb_d.ap())
            qb = wpool.tile([128, 1], f32, tag="qb", name="qb")
            nc.scalar.dma_start(qb[:], qb_d.ap())
            gpk = wpool.tile([128, 2 * K], f32, tag="gpk", name="gpk")
            nc.scalar.dma_start(gpk[:], gpk_d.ap())
            scratch = wpool.tile([32, 1], f32, tag="scr", name="scr")
            nc.gpsimd.memset(scratch[:], 0.0)
            warm_sb = wpool.tile([32, 512], bf16, tag="warm", name="warm")
            nc.gpsimd.memset(warm_sb[:], 0.0)

            # Pool queue: wk + xvA ci0 (Pool engine-time is precious)
            wk = wpool.tile([128, 2, CM], bf16, tag="wk", name="wk")
            nc.gpsimd.dma_start(wk[:], wk_d.ap())
            nc.gpsimd.dma_start(xvA[:][:, 0], xp_d.ap()[:, 0, 0:XSPLIT])
            # wq on Pool (needed by the qm matmuls ~4.5us)
            wq = wpool.tile([128, 2, 128], bf16, tag="wq", name="wq")
            nc.gpsimd.dma_start(wq[:], wq_d.ap())

            # SP queue: xv ci1 chunks + xvB ci0, sm, then remaps.
            nc.sync.dma_start(xvA[:][:, 1], xp_d.ap()[:, 1, 0:XSPLIT])
            nc.sync.dma_start(xvB[:][:, 0], xp_d.ap()[:, 0, XSPLIT:])
            nc.sync.dma_start(xvB[:][:, 1], xp_d.ap()[:, 1, XSPLIT:])
            smat = wpool.tile([128, 9, 128], bf16, tag="sm", name="sm")

            # xq value-view remaps read DRAM directly -> no deps, issue early
            xq = [[None] * 4 for _ in range(4)]  # [grp*2+ci][mslot]

            def emit_xq(grp, ci, mslot):
                base = 0 if grp == 0 else 4
                t = kq.tile([128, RH, WP], bf16,
                            tag=f"xq{grp}{ci}{mslot}",
                            name=f"xq{grp}{ci}{mslot}")
                src0 = xp_d.ap()[mslot * 32:(mslot + 1) * 32, ci]
                part = list(src0.ap[0])
                src = AP(tensor=src0.tensor,
                         offset=src0.offset + base * WP,
                         ap=[part, [WP, 4], [WP, RH], [1, WP]])
                nc.sync.dma_start(t[:], src)
                xq[grp * 2 + ci][mslot] = t

            fb = wpool.tile([128, 2], f32, tag="fb", name="fb")

            # ---------------- km32 / qm4 matmuls -------------------------
            # three overlapping row-range bf16 copies of km:
            #   aa = rows 0-9   (feeds km4(0,0))
            #   ab = rows 4-13  (feeds km4(1,0))
            #   b  = rows 7-20  (feeds km4(0,1) and km4(1,1))
            km32aa = kq.tile([CM, 10, WP], bf16, tag="km32aa", name="km32aa")
            km32ab = kq.tile([CM, 10, WP], bf16, tag="km32ab", name="km32ab")
            km32b = kq.tile([CM, 2 * K, WP], bf16, tag="km32b", name="km32b")
            aa_f = km32aa[:].rearrange("p r w -> p (r w)")
            ab_f = km32ab[:].rearrange("p r w -> p (r w)")
            b_f = km32b[:].rearrange("p r w -> p (r w)")
            qm4h = [kq.tile([128, K, WP], bf16, tag=f"qm4h{h}",
                            name=f"qm4h{h}") for h in range(2)]
            xvA_f = xvA[:].rearrange("p c r w -> p (c r w)")
            xvB_f = xvB[:].rearrange("p c r w -> p (c r w)")
            NA = XSPLIT * WP           # 682
            NB = (RHP - XSPLIT) * WP   # 620

            # PE p-state warmup: junk matmuls on zeroed SBUF ramp the clock
            warm_ps = psMM.tile([128, 512], f32, tag="mm", name="warm")
            for _ in range(5):
                nc.tensor.matmul(warm_ps[:], warm_sb[:, 0:128], warm_sb[:],
                                 start=True, stop=True)
            wjunk = wpool.tile([1, 8], f32, tag="wj", name="wj")
            nc.scalar.copy(wjunk[:], warm_ps[:1, :8])

            def km_chunk(name, src_f, off, n):
                ps = psMM.tile([128, 512], f32, tag="mm", name=name)
                for ci in range(2):
                    nc.tensor.matmul(
                        ps[:CM, :n],
                        wk[:, ci], src_f[:, ci * (NA if src_f is xvA_f else NB)
                                         + off: ci * (NA if src_f is xvA_f
                                                      else NB) + off + n],
                        start=(ci == 0), stop=(ci == 1))
                return ps

            ps_c0 = km_chunk("km_c0", xvA_f, 0, 434)        # rows 0-6
            ps_c1 = km_chunk("km_c1", xvA_f, 434, 248)      # rows 7-10
            # aa copies (rows 0-9) -> km4(0,0) can go as soon as these land
            nc.scalar.activation(aa_f[:, 0:434], ps_c0[:CM, :434],
                                 Ident, bias=kb[:], scale=1.0)
            nc.scalar.activation(aa_f[:, 434:620], ps_c1[:CM, :186],
                                 Ident, bias=kb[:], scale=1.0)
            ps_c2 = km_chunk("km_c2", xvB_f, 0, 434)        # rows 11-17
            # ab copies (rows 4-13)
            nc.scalar.activation(ab_f[:, 0:186], ps_c0[:CM, 248:434],
                                 Ident, bias=kb[:], scale=1.0)
            nc.scalar.activation(ab_f[:, 186:434], ps_c1[:CM, :248],
                                 Ident, bias=kb[:], scale=1.0)
            nc.scalar.activation(ab_f[:, 434:620], ps_c2[:CM, :186],
                                 Ident, bias=kb[:], scale=1.0)
            ps_c3 = km_chunk("km_c3", xvB_f, 434, 186)      # rows 18-20
            # b copies (rows 7-20)
            nc.scalar.activation(b_f[:, 0:248], ps_c1[:CM, :248],
                                 Ident, bias=kb[:], scale=1.0)
            nc.scalar.activation(b_f[:, 248:682], ps_c2[:CM, :434],
                                 Ident, bias=kb[:], scale=1.0)
            nc.scalar.activation(b_f[:, 682:868], ps_c3[:CM, :186],
                                 Ident, bias=kb[:], scale=1.0)
            # qm chunks after all km chunks (wq arrives later than wk)
            psq0 = psMM.tile([128, 512], f32, tag="mm", name="psq0")
            for ci in range(2):
                nc.tensor.matmul(psq0[:, :434],
                                 wq[:, ci],
                                 xvA_f[:, ci * NA + PAD * WP:
                                       ci * NA + PAD * WP + 434],
                                 start=(ci == 0), stop=(ci == 1))
            psq1 = psMM.tile([128, 512], f32, tag="mm", name="psq1")
            for ci in range(2):
                nc.tensor.matmul(psq1[:, 0:62],
                                 wq[:, ci], xvA_f[:, ci * NA + 620:
                                                  ci * NA + 682],
                                 start=(ci == 0), stop=False)
                nc.tensor.matmul(psq1[:, 62:434],
                                 wq[:, ci], xvB_f[:, ci * NB: ci * NB + 372],
                                 start=False, stop=(ci == 1))

            # exp-table preload once the Act queue head has drained
            nc.scalar.activation(scratch[:], scratch[:], Exp, bias=0.0,
                                 scale=1.0)

            # qm bias-copies on DVE
            nc.vector.tensor_scalar_add(
                qm4h[0][:].rearrange("p r w -> p (r w)"),
                psq0[:, :434], qb[:])
            nc.vector.tensor_scalar_add(
                qm4h[1][:].rearrange("p r w -> p (r w)"),
                psq1[:, :434], qb[:])

            # km4 remap DMAs, one tile per (grp, half)
            km4h = [[kq.tile([128, K, WP], bf16, tag=f"km4{g}{h}",
                             name=f"km4{g}{h}") for h in range(2)]
                    for g in range(2)]

            def emit_km4(grp, h, queue):
                # source tile and local base row for each (grp, h):
                # (0,0)->aa row 0; (1,0)->ab row 0; (0,1)->b row 0;
                # (1,1)->b row 4
                srcs = {(0, 0): (km32aa, 0), (1, 0): (km32ab, 0),
                        (0, 1): (km32b, 0), (1, 1): (km32b, 4)}
                tile_src, base = srcs[(grp, h)]
                a = tile_src[:]
                part = list(a.ap[0])
                src = AP(tensor=a.tensor, offset=a.offset + base * WP,
                         ap=[part, [WP, 4], [WP, K], [1, WP]])
                queue.dma_start(km4h[grp][h][:], src)

            emit_km4(0, 0, nc.sync)
            emit_km4(1, 0, nc.sync)
            emit_km4(0, 1, nc.sync)
            emit_km4(1, 1, nc.sync)
            nc.sync.dma_start(smat[:], sm_d.ap())
            nc.sync.dma_start(fb[:], fb_d.ap())
            # xq remaps (after the km4 remaps on SP)
            for grp, h, jobs in BLOCKS[:2]:
                for (ci, m, eng, planes) in jobs:
                    if xq[grp * 2 + ci][m] is None:
                        emit_xq(grp, ci, m)

            # fw needed only for the final conv
            fw = wpool.tile([128, 4, 128], bf16, tag="fw", name="fw")
            nc.sync.dma_start(fw[:], fw_d.ap())

            ident = smat[:, 8]               # [128, 128] identity

            # ---------------- attention (packed, per (grp, half)) --------
            P4h = [[att.tile([128, K, K, W], bf16, tag=f"P4{g}{h}",
                             name=f"P4{g}{h}") for h in range(2)]
                   for g in range(2)]
            E4h = [[att.tile([128, K, 392], bf16, tag=f"E4{g}{h}",
                             name=f"E4{g}{h}") for h in range(2)]
                   for g in range(2)]
            dps = [[None, None] for _ in range(2)]
            rbh = [[att.tile([128, 392], bf16, tag=f"rb{g}{h}",
                             name=f"rb{g}{h}") for h in range(2)]
                   for g in range(2)]
            ck4h = [[att.tile([128, K, 392], bf16, tag=f"ck{g}{h}",
                              name=f"ck{g}{h}") for h in range(2)]
                    for g in range(2)]

            def emit_P4(grp, h):
                nsplit = P4_NSPLIT[(grp, h)]
                qmc = qm4h[h][:][:, :, PAD:PAD + W]
                for kx in range(K):
                    eng = nc.vector if kx < nsplit else nc.gpsimd
                    eng.tensor_mul(
                        P4h[grp][h][:, kx],
                        km4h[grp][h][:][:, :, kx:kx + W],
                        qmc)

            def emit_exps(grp, h):
                for kx in range(K):
                    nc.scalar.activation(
                        E4h[grp][h][:, kx],
                        P4h[grp][h][:, kx].rearrange("p r w -> p (r w)"),
                        Exp,
                        bias=gpk[:, grp * K + kx:grp * K + kx + 1],
                        scale=1.0)

            def emit_dsum(grp, h):
                ps = psB.tile([128, 392], f32, tag="pso", name=f"d{grp}{h}")
                dps[grp][h] = ps
                for kx in range(K):
                    nc.tensor.matmul(ps[:], ident, E4h[grp][h][:, kx],
                                     start=(kx == 0), stop=(kx == K - 1))

            def emit_norm(grp, h):
                from concourse.dve_ops import (RECIPROCAL_APPROX_FAST,
                                               RECIP_APPROX_FAST_CONSTS)
                dsrc = dps[grp][h][:]
                nc.vector._custom_dve(RECIPROCAL_APPROX_FAST,
                                      out=rbh[grp][h][:], in0=dsrc,
                                      **RECIP_APPROX_FAST_CONSTS)
                ne = NORM_ENG[(grp, h)]
                if ne == 's':   # split across both vector engines
                    nc.vector.tensor_mul(
                        ck4h[grp][h][:][:, 0:4], E4h[grp][h][:][:, 0:4],
                        rbh[grp][h][:].unsqueeze(1).broadcast_to((128, 4, 392)))
                    nc.gpsimd.tensor_mul(
                        ck4h[grp][h][:][:, 4:7], E4h[grp][h][:][:, 4:7],
                        rbh[grp][h][:].unsqueeze(1).broadcast_to((128, 3, 392)))
                else:
                    neng = nc.vector if ne == 'v' else nc.gpsimd
                    neng.tensor_mul(
                        ck4h[grp][h][:], E4h[grp][h][:],
                        rbh[grp][h][:].unsqueeze(1).broadcast_to((128, K, 392)))

            # attention phases in value-block order
            for (grp, h) in [(0, 0), (1, 0), (0, 1), (1, 1)]:
                emit_P4(grp, h)
                emit_exps(grp, h)
            emit_dsum(0, 0)
            if (1, 0) not in DSUM_INJECT.values():
                emit_dsum(1, 0)
            emit_norm(0, 0)

            # ---------------- value phase --------------------------------
            pre_ps = [[psA.tile([128, n], f32, tag=f"pre{ci}{si}",
                                name=f"pre{ci}{si}")
                       for si, (o, n) in enumerate(SPLITS)] for ci in range(2)]
            first = [[True] * 2 for _ in range(2)]
            # last (ci, h) job position for stop flags
            last_pos = {}
            for bi, (grp, h, jobs) in enumerate(BLOCKS):
                for ji, (ci, m, eng, planes) in enumerate(jobs):
                    last_pos[(ci, h)] = (bi, ji)

            def emit_job(bi, ji, grp, h, ci, m, eng_c, planes):
                PV = pvp.tile([128, K, 392], bf16, tag="PV",
                              name=f"PV{bi}_{ji}")
                eng = nc.vector if eng_c == 'v' else nc.gpsimd
                xw = xq[grp * 2 + ci][m][:]
                part = list(xw.ap[0])
                win = AP(tensor=xw.tensor,
                         offset=xw.offset + (7 * WP if h else 0),
                         ap=[part, [1, K], [WP, K], [1, W]])
                eng.tensor_mul(
                    PV[:].rearrange("p k (r w) -> p k r w", r=K),
                    ck4h[grp][h][:].rearrange("p k (r w) -> p k r w", r=K),
                    win)
                S_ap = smat[:, grp * 4 + m]
                if planes == 4:
                    # single-inst 7->4 presum: PW[j] = PV[j] + PV[j+3]
                    PW = pvw.tile([128, 3, 392], bf16, tag="PW",
                                  name=f"PW{bi}_{ji}")
                    eng.tensor_add(PW[:], PV[:, 0:3], PV[:, 3:6])
                    srcs = [PW[:, 0, :], PW[:, 1, :], PW[:, 2, :],
                            PV[:, 6, :]]
                else:
                    srcs = [PV[:, kx, :] for kx in range(K)]
                for kx, s in enumerate(srcs):
                    is_last = (last_pos[(ci, h)] == (bi, ji)
                               and kx == len(srcs) - 1)
                    nc.tensor.matmul(pre_ps[ci][h][:], S_ap, s,
                                     start=first[ci][h], stop=is_last)
                    first[ci][h] = False

            # ---------------- final 1x1 conv (per half) ------------------
            y_sb = outp.tile([128, 2, L], bf16, tag="ysb", name="ysb")
            pre_sb = [[None] * 2 for _ in range(2)]

            def emit_conv(h):
                o, n = SPLITS[h]
                for ci in range(2):
                    t = outp.tile([128, 392], bf16, tag=f"psb{ci}{h}",
                                  name=f"psb{ci}{h}")
                    pre_sb[ci][h] = t
                    # h1: ci1's accumulator stops last -> faster Act copy
                    on_act = (ci == 1) if h == 1 else (ci == 0 or CONV_H0_ACT)
                    if on_act:
                        nc.scalar.copy(t[:], pre_ps[ci][h][:])
                    else:
                        nc.vector.tensor_scalar_mul(t[:], pre_ps[ci][h][:],
                                                    1.0)
                # two column pieces per oc: bias+DMA of piece 0 overlaps the
                # rest so only a small piece trails the final matmul.
                pieces = CONV_PIECES if h == 1 else ((0, 392),)
                for oc in range(2):
                    ps = psB.tile([128, 512], f32, tag="pso",
                                  name=f"ps_o{oc}{h}")
                    for ci in range(2):
                        nc.tensor.matmul(ps[:, :n], fw[:, 2 * ci + oc],
                                         pre_sb[ci][h][:],
                                         start=(ci == 0), stop=(ci == 1))
                    for (po, pn) in pieces:
                        # h1: oc1 is gated later, so it gets the faster Act
                        # bias; oc0 takes DVE.  h0 keeps the original split.
                        oc_on_act = (oc == 1) if h == 1 else (oc == 0)
                        if oc_on_act:
                            nc.scalar.activation(
                                y_sb[:, oc, o + po:o + po + pn],
                                ps[:, po:po + pn], Ident,
                                bias=fb[:, oc:oc + 1], scale=1.0)
                            nc.sync.dma_start(
                                y_d.ap()[:, oc, o + po:o + po + pn],
                                y_sb[:][:, oc, o + po:o + po + pn])
                        else:
                            nc.vector.tensor_scalar_add(
                                y_sb[:, oc, o + po:o + po + pn],
                                ps[:, po:po + pn], fb[:, oc:oc + 1])
                            nc.scalar.dma_start(
                                y_d.ap()[:, oc, o + po:o + po + pn],
                                y_sb[:][:, oc, o + po:o + po + pn])

            for bi, (grp, h, jobs) in enumerate(BLOCKS):
                if bi == 2:
                    # grp-1 xq tiles emitted once block A0's are all queued
                    for g2, h2, jobs2 in BLOCKS[2:]:
                        for (ci, m, eng, planes) in jobs2:
                            if xq[g2 * 2 + ci][m] is None:
                                emit_xq(g2, ci, m)
                for ji, (ci, m, eng_c, planes) in enumerate(jobs):
                    emit_job(bi, ji, grp, h, ci, m, eng_c, planes)
                    if (bi, ji) in DSUM_INJECT:
                        g2, h2 = DSUM_INJECT[(bi, ji)]
                        emit_dsum(g2, h2)
                    if (bi, ji) in NORM_INJECT:
                        g2, h2 = NORM_INJECT[(bi, ji)]
                        emit_norm(g2, h2)
                # end of blocks 1 and 3 complete halves 0 and 1
                if bi == 1:
                    emit_conv(0)
                if bi == 3:
                    emit_conv(1)

    nc.compile()
    return nc


def _get_program():
    global _PROGRAM
    if _PROGRAM is None:
        _PROGRAM = _build_program()
    return _PROGRAM


def _gpk_host(gp_w1, gp_b1, gp_w2, gp_b2):
    """GeometryPrior on host (tiny: 49 positions through a 2->16->32 MLP)."""
    a = np.arange(-(K // 2), K // 2 + 1, dtype=np.float32)
    x_pos = np.broadcast_to(a[None, :], (K, K))
    y_pos = np.broadcast_to(a[::-1][:, None], (K, K))
    pos = np.stack([x_pos, y_pos], 0).astype(np.float32)          # [2,7,7]
    h1 = np.einsum('pij,mp->mij', pos, np.asarray(gp_w1, np.float32))
    h1 = np.maximum(h1 + np.asarray(gp_b1, np.float32)[:, None, None], 0.0)
    gpk = np.einsum('mij,cm->cij', h1, np.asarray(gp_w2, np.float32))
    gpk = gpk + np.asarray(gp_b2, np.float32)[:, None, None]      # [32,7,7]
    return gpk


def make_inputs(x, k_w, k_b, q_w, q_b, gp_w1, gp_b1, gp_w2, gp_b2, f_w, f_b):
    """Returns per-core input maps (list of 8 dicts)."""
    x = np.asarray(x, np.float32)
    xp = np.zeros((B, C, HP, WP), np.float32)
    xp[:, :, PAD:PAD + H, PAD:PAD + W] = x

    # channel order: chunk ci partition p -> c = (4ci + p//32)*32 + p%32
    chan = np.arange(128)
    c_of = [((4 * ci + chan // 32) * 32 + chan % 32) for ci in range(2)]

    k_w = np.asarray(k_w, np.float32)
    q_w = np.asarray(q_w, np.float32)
    f_w = np.asarray(f_w, np.float32)

    # wk[p, ci, cm] = k_w[cm, c_of[ci][p]]
    wk = np.stack([k_w[:, c_of[ci]].T for ci in range(2)], 1).astype(_bf16)
    # wq[p, ci, cm*4+g] = q_w[cm, c_of[ci][p]]
    wq_rows = np.stack([q_w[:, c_of[ci]].T for ci in range(2)], 1)  # [128,2,32]
    wq = np.repeat(wq_rows, 4, axis=2)                              # cm*4+g
    wq = wq.reshape(128, 2, CM, 4).reshape(128, 2, 128).astype(_bf16)

    # selection matrices: sm[k, grp*4+mslot, q] = 1 iff k = (q%32)*4+g valid g
    # slot 8 is the identity (for the softmax-denominator sums on PE)
    sm = np.zeros((128, 9, 128), np.float32)
    for grp in range(2):
        ng = 4 if grp == 0 else 3
        for mslot in range(4):
            for cm in range(CM):
                for g in range(ng):
                    sm[cm * 4 + g, grp * 4 + mslot, mslot * 32 + cm] = 1.0
    sm[:, 8, :] = np.eye(128, dtype=np.float32)
    sm = sm.astype(_bf16)

    # fw[p, 2*ci+oc, q] = f_w[c_of[oc][q], c_of[ci][p]]
    fw = np.zeros((128, 4, 128), np.float32)
    for ci in range(2):
        for oc in range(2):
            fw[:, 2 * ci + oc, :] = f_w[np.ix_(c_of[oc], c_of[ci])].T
    fw = fw.astype(_bf16)

    gpk_full = _gpk_host(gp_w1, gp_b1, gp_w2, gp_b2)      # [32,7,7]
    gpk = np.zeros((128, 2 * K), np.float32)
    for cm in range(CM):
        for g in range(4):
            gpk[cm * 4 + g, 0:K] = gpk_full[cm, g]
            gpk[cm * 4 + g, K:2 * K] = gpk_full[cm, 4 + min(g, 2)]

    kb = np.asarray(k_b, np.float32).reshape(CM, 1)
    qb = np.repeat(np.asarray(q_b, np.float32), 4).reshape(128, 1)
    fb = np.stack([np.asarray(f_b, np.float32)[c_of[oc]] for oc in range(2)],
                  1)                                       # [128, 2]

    xp16 = xp.astype(_bf16)
    in_maps = []
    for core in range(NCORES):
        b, rb = divmod(core, RB)
        sl = np.zeros((128, 2, RHP, WP), _bf16)
        r0 = rb * RH
        nrows = min(RHP, HP - r0)
        for ci in range(2):
            sl[:, ci, :nrows] = xp16[b, c_of[ci], r0:r0 + nrows, :]
        in_maps.append({
            "xp": sl, "wk": wk, "wq": wq, "sm": sm, "fw": fw, "gpk": gpk,
            "kb": kb, "qb": qb, "fb": fb,
        })
    return in_maps


def assemble(results):
    chan = np.arange(128)
    out = np.empty((B, C, H, W), np.float32)
    for core in range(NCORES):
        b, rb = divmod(core, RB)
        y = np.asarray(results[core]["y"], np.float32)     # [128, 2, 784]
        for oc in range(2):
            c_of = (4 * oc + chan // 32) * 32 + chan % 32
            out[b, c_of, rb * RH:(rb + 1) * RH, :] = y[:, oc].reshape(
                128, RH, W)
    return out


def kernel(**inputs):
    from concourse import bass_utils
    nc = _get_program()
    in_maps = make_inputs(**inputs)
    res = bass_utils.run_bass_kernel_spmd(nc, in_maps, list(range(NCORES)))
    return assemble(res.results)
